# revision 62
# baseline (speedup 1.0000x reference)
"""Trainium2 Bass kernel: MultiHeadSelfAttention (LayerNorm -> QKV -> masked
softmax attention -> output projection).

Problem shapes: B=4, S=2048, D=512, H=8, DK=64, fp32 I/O.

Sharding: 8 cores = 4 batches x 2 query-halves. Each core computes the full
K/V for its batch and attention outputs for its 1024 queries; no cross-core
communication. SPMD trick: the token order of each core's input is permuted on
the host so that the core's queries are always tokens 0..1023 (one static
program for all cores; attention is permutation-equivariant over keys as long
as the key-padding mask is permuted consistently).

Host-side prep (elementwise/layout only; all matmuls + softmax on device):
LayerNorm-normalize, fold gamma/beta into weights/biases, cast to bf16, and
lay out xnT d-major [128, half, dc, tok] so the device DMAs it straight into
SBUF with no on-device LN or transposes.

Device dataflow (single software-pipelined program; the ACT exp stream — 128
activations of [128,1024], ~134us — is the critical path and everything else
hides under it):
  - warmup: sliced DMAs land only what pair 0 needs first; dummy ident
    matmuls ramp the PE p-state; a dummy exp preloads the ACT table set.
    Then Q projection (d-major qT, pair-0 columns), K dkc0, V chunks 0-1.
  - one uniform stream over (pair, chunk) steps, PV lagging one step: per
    key chunk c (16 x 128 keys): scoresT[k,q] via PE (contract DK=64), one
    ACT op per head half fusing scale 1/8 + additive key-padding mask bias
    + exp into bf16 pt (softmax without max-subtraction is safe here:
    |scores| <~ 8); PV accumulates P@[V_h|1] into 3 packed PSUM banks (the
    ones column yields the softmax denominator l for free).
    PE filler slotted into the chunk loop (heavy K/Q projections strictly
    alternating with light V slots, deadline-ordered); pair p-1's
    attention-output transposes ride the idle HWDGE DMA xbar.
  - attno evacuation per pair: batched reciprocal of l, scale by 1/l. The
    first two PV chunks of each new pair are deferred one step so the PV
    banks' evacuation (DVE) never stalls the in-order PE queue.
  - pair 3 has no next-pair filler work, so its spare slots compute
    output-projection partials over d-chunks 0-2 into bf16 SBUF (yacc).
  - last chunk of the last pair is emitted per PSUM bank group: PV, evac
    (split across DVE and the now-idle ACT), transpose, then that group's
    output projections immediately (one d-chunk-3 matmul + an
    identity-matmul add of yacc) — every PSUM bank hosts an O-proj tile so
    the tail drains fast. y is written bf16; the output-projection bias is
    applied on the host (elementwise) along with the f32 upcast.

PSUM budget (8 banks): scp 2x[128,1024]f32 (scores ping/pong; warmup
projections and tail O-proj reuse it) = 4, pvp 3x[128,512]f32 = 3,
filp 1x[128,512]f32 (filler V/K/O-partial proj, tail transposes) = 1.
"""

import math

import numpy as np

import concourse.bass as bass
import concourse.tile as tile
from concourse import bacc, mybir
from concourse.bass_utils import run_bass_kernel_spmd
from concourse.masks import make_identity

B, S, D, H, DK = 4, 2048, 512, 8, 64
P = 128                 # partitions
NQ = 1024               # queries per core
NT = S // P             # 16 token tiles / key chunks
DC = D // P             # 4 d-chunks
NQT = NQ // P           # 8 query tiles
PAIRS = H // 2          # 4 head pairs
F32 = mybir.dt.float32
BF16 = mybir.dt.bfloat16
NEG = -1.0e30


def _ap(sl, dims):
    """AP over slice `sl` (a [P,1] slice): partition dim + given free dims."""
    return bass.AP(tensor=sl.tensor, offset=sl.offset, ap=[sl.ap[0]] + dims)


def _emit(tc: tile.TileContext, ctx):
    nc = tc.nc

    xnt_d = nc.dram_tensor("xnt", [P, 2 * DC * NQ], BF16, kind="ExternalInput")
    wq_d = nc.dram_tensor("wq", [P, DC * D], BF16, kind="ExternalInput")
    wk_d = nc.dram_tensor("wk", [P, DC * D], BF16, kind="ExternalInput")
    wv_d = nc.dram_tensor("wv", [P, DC * D], BF16, kind="ExternalInput")
    wo_d = nc.dram_tensor("wo", [P, DC * D], BF16, kind="ExternalInput")
    bq_d = nc.dram_tensor("bq", [P, DC], F32, kind="ExternalInput")
    bk_d = nc.dram_tensor("bk", [P, DC], F32, kind="ExternalInput")
    mb_d = nc.dram_tensor("maskb", [P, NT], F32, kind="ExternalInput")
    y_d = nc.dram_tensor("y", [NQ, D], BF16, kind="ExternalOutput")

    consts = ctx.enter_context(tc.tile_pool(name="consts", bufs=1))
    big = ctx.enter_context(tc.tile_pool(name="big", bufs=1))
    ptp = ctx.enter_context(tc.tile_pool(name="ptp", bufs=4))
    rlp = ctx.enter_context(tc.tile_pool(name="rlp", bufs=4))
    yout = ctx.enter_context(tc.tile_pool(name="yout", bufs=8))

    ident = consts.tile([P, P], BF16, tag="ident")
    make_identity(nc, ident)
    mb_sb = consts.tile([P, NT], F32, tag="mb")
    bq_sb = consts.tile([P, DC], F32, tag="bq")
    bk_sb = consts.tile([P, DC], F32, tag="bk")

    # persistent bf16 operands (DMA'd directly, no casts)
    xnT = big.tile([P, 2, DC, NQ], BF16, tag="xnT")
    # wq/wk: [P, out-block(dqc/dkc), dc, 128]; wv: [P, pair, dc, 128];
    # wo: [P, dc, 512]. Block-major so the warmup needs one contiguous DMA.
    w_sb = {}
    for name in ("wq", "wk", "wv"):
        w_sb[name] = big.tile([P, DC, DC, P], BF16, tag=f"{name}_sb", name=f"{name}_sb")
    w_sb["wo"] = big.tile([P, DC, D], BF16, tag="wo_sb", name="wo_sb")
    qT = big.tile([P, DC, NQ], BF16, tag="qT")
    kT = big.tile([P, DC, S], BF16, tag="kT")
    vaug = big.tile([P, NT, H * 65], BF16, tag="vaug")
    attno = big.tile([P, NQT, D], BF16, tag="attno")
    outT = big.tile([P, DC, NQ], BF16, tag="outT")
    yacc = big.tile([P, NQT, D], BF16, tag="yacc")

    # Serial-DMA byte order = need order: only the column slices required by
    # the warmup go first (wq dqc0, xnT h0, wk dkc0, wv pair0), bulk later,
    # tail-only bo last.
    def w_blk(name, d_tensor, lo, hi):
        nc.sync.dma_start(
            w_sb[name][:, lo:hi].rearrange("p b c j -> p (b c j)"),
            d_tensor[:, lo * D : hi * D],
        )

    w_blk("wq", wq_d, 0, 1)
    for dc in range(DC):
        nc.sync.dma_start(xnT[:, 0, dc, :], xnt_d[:, dc * NQ : (dc + 1) * NQ])
    nc.sync.dma_start(mb_sb, mb_d[:, :])
    nc.sync.dma_start(bq_sb, bq_d[:, :])
    nc.sync.dma_start(bk_sb, bk_d[:, :])
    w_blk("wk", wk_d, 0, 1)
    w_blk("wv", wv_d, 0, 1)
    nc.sync.dma_start(
        xnT[:, 1].rearrange("p c t -> p (c t)"), xnt_d[:, DC * NQ : 2 * DC * NQ]
    )
    w_blk("wq", wq_d, 1, DC)
    w_blk("wk", wk_d, 1, DC)
    w_blk("wv", wv_d, 1, DC)
    nc.sync.dma_start(w_sb["wo"][:].rearrange("p c d -> p (c d)"), wo_d[:, :])

    # Preload the exp table set while the DMAs run (dummy 1-element exp).
    dummy = consts.tile([P, 1], F32, tag="dummy")
    nc.vector.memset(dummy, 0.0)
    dummy_o = consts.tile([P, 1], BF16, tag="dummy_o")
    nc.scalar.activation(
        out=dummy_o, in_=dummy, func=mybir.ActivationFunctionType.Exp
    )

    # ones columns of vaug (one strided memset per token chunk)
    for t in range(NT):
        nc.vector.memset(_ap(vaug[:, t, DK : DK + 1], [[65, H]]), 1.0)

    scp = ctx.enter_context(tc.tile_pool(name="scp", bufs=2, space="PSUM"))
    pvp = ctx.enter_context(tc.tile_pool(name="pvp", bufs=3, space="PSUM"))
    filp = ctx.enter_context(tc.tile_pool(name="filp", bufs=1, space="PSUM"))

    def xn_mv(half, dc, lo, n):
        return xnT[:, half, dc, lo : lo + n]

    def q_proj(dqc, qg, pool, act=False):
        t = (
            pool.tile([P, NQ], F32, tag="sc", name=f"q{dqc}_{qg}")
            if pool is scp
            else pool.tile([P, 512], F32, tag="fil", name=f"qf{dqc}_{qg}")
        )
        ps = t[:, 0:512]
        for dc in range(DC):
            nc.tensor.matmul(
                ps,
                w_sb["wq"][:, dqc, dc, :],
                xn_mv(0, dc, qg * 512, 512),
                start=(dc == 0), stop=(dc == DC - 1),
            )
        dst = qT[:, dqc, qg * 512 : (qg + 1) * 512]
        if act:
            nc.scalar.activation(
                out=dst, in_=ps, func=mybir.ActivationFunctionType.Identity,
                bias=bq_sb[:, dqc : dqc + 1],
            )
        else:
            nc.vector.tensor_scalar_add(
                out=dst, in0=ps, scalar1=bq_sb[:, dqc : dqc + 1]
            )

    def k_proj(dkc, kg, pool, act=False):
        t = (
            pool.tile([P, NQ], F32, tag="sc", name=f"k{dkc}_{kg}")
            if pool is scp
            else pool.tile([P, 512], F32, tag="fil", name=f"kf{dkc}_{kg}")
        )
        ps = t[:, 0:512]
        for dc in range(DC):
            nc.tensor.matmul(
                ps,
                w_sb["wk"][:, dkc, dc, :],
                xn_mv(kg // 2, dc, (kg % 2) * 512, 512),
                start=(dc == 0), stop=(dc == DC - 1),
            )
        dst = kT[:, dkc, kg * 512 : (kg + 1) * 512]
        if act:
            nc.scalar.activation(
                out=dst, in_=ps, func=mybir.ActivationFunctionType.Identity,
                bias=bk_sb[:, dkc : dkc + 1],
            )
        else:
            nc.vector.tensor_scalar_add(
                out=dst, in0=ps, scalar1=bk_sb[:, dkc : dkc + 1]
            )

    def v_proj2(p, c0, pool, nch=2):
        """V projection for chunks c0..c0+nch-1, pair p's 128 head dims."""
        t = (
            pool.tile([P, NQ], F32, tag="sc", name=f"v{p}_{c0}")
            if pool is scp
            else pool.tile([P, 512], F32, tag="fil", name=f"vf{p}_{c0}")
        )
        for i in range(nch):
            for dc in range(DC):
                nc.tensor.matmul(
                    t[:, i * P : i * P + P],
                    xn_mv((c0 + i) // 8, dc, ((c0 + i) % 8) * P, P),
                    w_sb["wv"][:, p, dc, :],
                    start=(dc == 0), stop=(dc == DC - 1),
                )
        # [128 tok, nch*(2*64)] -> vaug slots [64 | skip l-col | 64] per chunk
        dst = _ap(
            vaug[:, c0, 2 * p * 65 : 2 * p * 65 + 1],
            [[H * 65, nch], [65, 2], [1, DK]],
        )
        nc.vector.tensor_copy(
            out=dst,
            in_=t[:, 0 : nch * P].rearrange("p (c h k) -> p c h k", h=2, k=DK),
        )

    def e_transpose_dma(p, qt0):
        """Transpose attention output via the idle HWDGE xbar (SBUF->SBUF)."""
        for qt in (qt0, qt0 + 1):
            nc.sync.dma_start(
                outT[:, p, qt * P : (qt + 1) * P],
                attno[:, qt, p * P : (p + 1) * P],
                transpose=True,
            )

    def e_transpose2(p, qt0, use_act=False):
        """PE-transpose attention output of pair p, query tiles qt0, qt0+1."""
        pe = filp.tile([P, 512], F32, tag="fil", name=f"e{p}_{qt0}")
        peb = pe[:, :].bitcast(BF16)
        for i in range(2):
            nc.tensor.transpose(
                peb[:, i * P : (i + 1) * P], attno[:, qt0 + i, p * P : (p + 1) * P],
                ident,
            )
        if use_act:
            nc.scalar.copy(outT[:, p, qt0 * P : (qt0 + 2) * P], peb[:, 0 : 2 * P])
        else:
            nc.vector.tensor_copy(
                out=outT[:, p, qt0 * P : (qt0 + 2) * P], in_=peb[:, 0 : 2 * P]
            )

    def o_partial(qt):
        """Output-projection partial over d-chunks 0-2 (pairs 0-2), rounded
        to bf16 in SBUF; the tail adds chunk 3 + this via the PE."""
        t = filp.tile([P, 512], F32, tag="fil", name=f"op{qt}")
        for dc in range(DC - 1):
            nc.tensor.matmul(
                t,
                outT[:, dc, qt * P : (qt + 1) * P],
                w_sb["wo"][:, dc, :],
                start=(dc == 0), stop=(dc == DC - 2),
            )
        nc.vector.tensor_copy(out=yacc[:, qt, :], in_=t)

    pts = {}
    fillers = {}
    pvbs = {}
    pv_pend = []

    def scores_exp(p, c, step):
        pt = ptp.tile([P, 2 * NQ], BF16, tag="pt", name=f"pt{p}_{c}")
        pts[step] = pt
        for hs in range(2):
            sc = scp.tile([P, NQ], F32, tag="sc", name=f"sc{p}_{c}_{hs}")
            for qg in range(2):
                nc.tensor.matmul(
                    sc[:, qg * 512 : (qg + 1) * 512],
                    kT[hs * DK : (hs + 1) * DK, p, c * P : (c + 1) * P],
                    qT[hs * DK : (hs + 1) * DK, p, qg * 512 : (qg + 1) * 512],
                    start=True, stop=True,
                )
            nc.scalar.activation(
                out=pt[:, hs * NQ : (hs + 1) * NQ], in_=sc,
                func=mybir.ActivationFunctionType.Exp,
                bias=mb_sb[:, c : c + 1], scale=1.0 / math.sqrt(DK),
            )

    # ---------------- warmup (pair-0 prerequisites only) ----------------
    # Prime the PE p-state with dummy ident matmuls (no DMA dependency), and
    # keep padding between the DMA-gated first projection's matmuls: any PE
    # idle gap resets the p-state ramp, so the engine must never starve.
    warm = filp.tile([P, 512], F32, tag="fil", name="warm")

    def pad(n):
        for _ in range(n):
            nc.tensor.matmul(warm[:, 0:P], ident, ident, start=True, stop=True)

    pad(6)
    t0 = scp.tile([P, NQ], F32, tag="sc", name="q0_0")
    for dc in range(DC):
        nc.tensor.matmul(
            t0[:, 0:512],
            w_sb["wq"][:, 0, dc, :],
            xn_mv(0, dc, 0, 512),
            start=(dc == 0), stop=(dc == DC - 1),
        )
        if dc < DC - 1:
            pad(3)
    nc.scalar.activation(
        out=qT[:, 0, 0:512], in_=t0[:, 0:512],
        func=mybir.ActivationFunctionType.Identity, bias=bq_sb[:, 0:1],
    )
    q_proj(0, 1, scp)
    pad(2)
    k_proj(0, 0, scp, act=True)
    pad(2)
    v_proj2(0, 0, scp)

    # Slot-scheduled filler work per pair (slot = key chunk index): the PE
    # runs these under the ACT-bound exp stream, ~1 PSUM-bank op per slot,
    # each placed just ahead of its deadline so the scores/exp pipeline
    # always has priority.
    def fillers_for(p):
        V = lambda pp, c0: (lambda: v_proj2(pp, c0, filp))
        K = lambda pp, kg: (lambda: k_proj(pp, kg, filp))
        Q = lambda pp, qg: (lambda: q_proj(pp, qg, filp))
        E = lambda pp, qt0: (lambda: e_transpose2(pp, qt0))
        sched = {}
        # Slot budget ~2.07us (one chunk's exp): scores+PV = ~1.3us. Heavy
        # ops (K/Q proj, 0.85) strictly alternate with light V slots (0.43).
        # E transposes ride the idle DMA xbar, costing no engine slot. K for
        # chunks 8-15 of pair p runs early IN pair p (deadline mid-pair).
        ED = lambda pp, qt0: (lambda: e_transpose_dma(pp, qt0))
        if p == 0:
            head = [(0, V(0, 2)), (2, V(0, 4)), (4, V(0, 6)), (6, V(0, 8)),
                    (8, V(0, 10)), (10, V(0, 12)), (12, V(0, 14)),
                    (1, K(0, 1)), (3, K(0, 2)), (5, K(0, 3))]
            nxt = [(7, K(1, 0)), (9, K(1, 1)), (11, Q(1, 0)), (13, Q(1, 1)),
                   (14, V(1, 0)), (15, V(1, 2))]
        else:
            head = [(0, V(p, 4)), (2, V(p, 6)), (4, V(p, 8)), (6, V(p, 10)),
                    (8, V(p, 12)), (10, V(p, 14)),
                    (1, K(p, 2)), (3, K(p, 3)),
                    (2, ED(p - 1, 0)), (4, ED(p - 1, 2)),
                    (6, ED(p - 1, 4)), (8, ED(p - 1, 6))]
            nxt = []
            if p < PAIRS - 1:
                nxt = [(5, K(p + 1, 0)), (7, K(p + 1, 1)),
                       (9, Q(p + 1, 0)), (11, Q(p + 1, 1)),
                       (12, V(p + 1, 0)), (13, V(p + 1, 2))]
            else:
                # no next pair: use the free slots for output-projection
                # partials over d-chunks 0-2 (bf16 in SBUF; tail adds chunk 3)
                OP = lambda qt: (lambda: o_partial(qt))
                nxt = [(5, OP(0)), (7, OP(1)), (9, OP(2)), (11, OP(3)),
                       (12, OP(4)), (13, OP(5)), (14, OP(6)), (15, OP(7))]
        for s, f in head + nxt:
            sched.setdefault(s, []).append(f)
        return sched

    def evac_bank(p, pvb, j, split_act=False):
        n = 2 * (3 if j < 2 else 2)
        rl = rlp.tile([P, 6], F32, tag="rl", name=f"rl{p}_{j}")
        nc.vector.reciprocal(
            out=rl[:, 0:n], in_=_ap(pvb[j][:, DK : DK + 1], [[65, n]])
        )
        for qt in range(3 * j, min(3 * j + 3, NQT)):
            off = (qt % 3) * 130
            r = (qt % 3) * 2
            for hs in range(2):
                dst = attno[:, qt, (2 * p + hs) * DK : (2 * p + hs + 1) * DK]
                srcb = pvb[j][:, off + hs * 65 : off + hs * 65 + DK]
                if split_act and hs == 1:
                    nc.scalar.mul(dst, srcb, rl[:, r + hs : r + hs + 1])
                else:
                    nc.vector.tensor_scalar_mul(
                        out=dst, in0=srcb, scalar1=rl[:, r + hs : r + hs + 1]
                    )

    def evac_pair(p, pvb):
        for j in range(3):
            evac_bank(p, pvb, j)

    # ---------------- attention: uniform (pair, chunk) stream ----------------
    # PV lags scores/exp by one step so the PE never waits on the current
    # chunk's exp; pair boundaries are seamless.

    for step in range(PAIRS * NT + 1):
        if step < PAIRS * NT:
            p, c = divmod(step, NT)
            if c == 0:
                fillers = fillers_for(p)
            scores_exp(p, c, step)
        if step > 0:
            sp, sc_ = divmod(step - 1, NT)
            if sc_ == 0:
                pvbs[sp] = [
                    pvp.tile([P, 512], F32, tag="pvb", name=f"pvb{sp}_{j}")
                    for j in range(3)
                ]
            # Defer the first two PV chunks of pairs 1-3 by one step: their
            # banks are still being evacuated (DVE) for the previous pair,
            # and a stalled PV would block the in-order PE queue right when
            # the next scores are due.
            if sp > 0 and sc_ in (0, 1):
                pv_pend.append((sp, sc_, step - 1))
                sc_ = None
            else:
                for xsp, xsc, xstep in pv_pend:
                    _pv_chunk(nc, pts.pop(xstep), vaug, pvbs[xsp], xsp, xsc)
                pv_pend = []
            if sc_ is None:
                pass
            elif sp == PAIRS - 1 and sc_ == NT - 1:
                # Last chunk of the last pair: per PSUM bank group, emit PV,
                # evacuate, transpose, and launch the output projections for
                # that group's query tiles immediately (all 8 PSUM banks are
                # free for them by construction).
                pvbx = pvbs.pop(sp)
                pt_last = pts.pop(step - 1)

                def o_proj(qt, po):
                    nc.tensor.matmul(
                        po,
                        outT[:, DC - 1, qt * P : (qt + 1) * P],
                        w_sb["wo"][:, DC - 1, :],
                        start=True, stop=False,
                    )
                    nc.tensor.matmul(
                        po, ident, yacc[:, qt, :], start=False, stop=True
                    )
                    yt = yout.tile([P, D], BF16, tag="yt", name=f"yt{qt}")
                    if qt % 2 == 0:
                        nc.vector.tensor_copy(out=yt, in_=po)
                    else:
                        nc.scalar.copy(yt, po)
                    nc.sync.dma_start(y_d[qt * P : (qt + 1) * P, :], yt)

                sct0 = scp.tile([P, NQ], F32, tag="sc", name="po01")
                sct1 = scp.tile([P, NQ], F32, tag="sc", name="po23")
                pos = [sct0[:, 0:512], sct0[:, 512:1024],
                       sct1[:, 0:512], sct1[:, 512:1024]]
                for j in range(3):
                    for qt in range(3 * j, min(3 * j + 3, NQT)):
                        for hs in range(2):
                            h = 2 * sp + hs
                            nc.tensor.matmul(
                                pvbx[j][:, (qt % 3) * 130 + hs * 65 : (qt % 3) * 130 + (hs + 1) * 65],
                                pt_last[:, hs * NQ + qt * P : hs * NQ + (qt + 1) * P],
                                vaug[:, sc_, h * 65 : (h + 1) * 65],
                                start=False, stop=True, skip_group_check=True,
                            )
                    evac_bank(sp, pvbx, j, split_act=True)
                    if j == 0:
                        e_transpose2(sp, 0, use_act=True)
                        o_proj(0, pos[0])
                        o_proj(1, pos[1])
                    elif j == 1:
                        e_transpose2(sp, 2)
                        e_transpose2(sp, 4, use_act=True)
                        o_proj(2, pos[2])
                        o_proj(3, pos[3])
                        o_proj(4, pvp.tile([P, 512], F32, tag="pvb", name="po4"))
                        o_proj(5, pvp.tile([P, 512], F32, tag="pvb", name="po5"))
                    else:
                        e_transpose2(sp, 6, use_act=True)
                        o_proj(6, pvp.tile([P, 512], F32, tag="pvb", name="po6"))
                        # scp gen-3 frees (after qt0/1 copies) ~2us before a
                        # 4th pvp generation would (after qt4's copy)
                        sct2 = scp.tile([P, NQ], F32, tag="sc", name="po7t")
                        o_proj(7, sct2[:, 0:512])
            else:
                _pv_chunk(nc, pts.pop(step - 1), vaug, pvbs[sp], sp, sc_)
                if sc_ == NT - 1:
                    evac_pair(sp, pvbs.pop(sp))
        if step < PAIRS * NT:
            for f in fillers.get(c, []):
                f()

def _pv_chunk(nc, pt, vaug, pvb, p, c):
    """P@[V|1] matmuls for chunk c of head-pair p: 8 query tiles x 2 heads,
    accumulated over chunks into the packed PSUM banks."""
    for qt in range(NQT):
        bank = pvb[qt // 3]
        off = (qt % 3) * 130
        for hs in range(2):
            h = 2 * p + hs
            # start=True clears has_written for the WHOLE bank, so only the
            # first packed region per bank may use it; the others rely on
            # overwrite-when-bit-clear for their first chunk.
            nc.tensor.matmul(
                bank[:, off + hs * 65 : off + (hs + 1) * 65],
                pt[:, hs * NQ + qt * P : hs * NQ + (qt + 1) * P],
                vaug[:, c, h * 65 : (h + 1) * 65],
                start=(c == 0 and qt % 3 == 0 and hs == 0),
                stop=(c == NT - 1),
                skip_group_check=True,
            )


_NC = None


def _get_nc():
    global _NC
    if _NC is None:
        from contextlib import ExitStack

        nc = bacc.Bacc(None, target_bir_lowering=False)
        with tile.TileContext(nc) as tc, ExitStack() as ctx:
            _emit(tc, ctx)
        nc.compile()
        _NC = nc
    return _NC


def kernel(
    inputs, input_lengths, pos_embed, ln_gamma, ln_beta,
    Wq, bq, Wk, bk, Wv, bv, Wo, bo,
):
    import ml_dtypes

    bf = ml_dtypes.bfloat16
    x = np.ascontiguousarray(np.asarray(inputs, np.float32))
    lengths = np.asarray(input_lengths, np.int32)
    g = np.asarray(ln_gamma, np.float32)
    be = np.asarray(ln_beta, np.float32)
    Wq = np.asarray(Wq, np.float32); bq = np.asarray(bq, np.float32)
    Wk = np.asarray(Wk, np.float32); bk = np.asarray(bk, np.float32)
    Wv = np.asarray(Wv, np.float32); bv = np.asarray(bv, np.float32)
    Wo = np.asarray(Wo, np.float32); bo = np.asarray(bo, np.float32)

    # Fold LayerNorm affine into the projections (exact: LN(x) = xh*g + be
    # with xh = (x-mu)*rstd, so LN(x)@W.T + b = xh@(g[:,None]*W.T) + (be@W.T + b)).
    def w_blocks(wh):
        # [in, out] -> [P, out-block, dc, 128] flattened (block-major cols)
        return np.ascontiguousarray(
            wh.reshape(DC, P, DC, P).transpose(1, 2, 0, 3).reshape(P, DC * D)
            .astype(bf)
        )

    wq_h = w_blocks(g[:, None] * Wq.T)
    wk_h = w_blocks(g[:, None] * Wk.T)
    wv_h = w_blocks(g[:, None] * Wv.T)
    wo_h = np.ascontiguousarray(
        Wo.T.reshape(DC, P, D).transpose(1, 0, 2).reshape(P, DC * D).astype(bf)
    )
    bq_h = np.ascontiguousarray((be @ Wq.T + bq).reshape(DC, P).T)
    bk_h = np.ascontiguousarray((be @ Wk.T + bk).reshape(DC, P).T)
    # V bias (incl. beta term) passes through softmax (rows sum to 1) and is
    # folded into the output-projection bias.
    bv_h = be @ Wv.T + bv
    bo_h = (bo + bv_h @ Wo.T).astype(np.float32)

    # Host LayerNorm-normalize (elementwise; affine already folded above),
    # then d-major layout [128 part, half, dc, tok] flattened per core.
    mu = x.mean(-1, keepdims=True)
    rstd = 1.0 / np.sqrt(x.var(-1, keepdims=True) + 1e-5)
    xn = ((x - mu) * rstd).astype(np.float32)

    maskb = np.where(np.arange(S)[None, :] < lengths[:, None], 0.0, NEG).astype(
        np.float32
    )

    nc = _get_nc()
    in_maps = []
    core_assign = []
    for b in range(B):
        xt = xn[b].T.reshape(DC, P, 2, NQ).transpose(1, 2, 0, 3).astype(bf)
        for gq in range(2):
            order = np.r_[gq * NQ : (gq + 1) * NQ, (1 - gq) * NQ : (2 - gq) * NQ]
            xh = xt if gq == 0 else xt[:, ::-1]
            in_maps.append(
                {
                    "xnt": np.ascontiguousarray(xh.reshape(P, 2 * DC * NQ)),
                    "wq": wq_h, "wk": wk_h, "wv": wv_h, "wo": wo_h,
                    "bq": bq_h, "bk": bk_h,
                    "maskb": np.ascontiguousarray(maskb[b][order].reshape(NT, P).T),
                }
            )
            core_assign.append((b, gq))

    global _LAST_IN_MAPS
    _LAST_IN_MAPS = in_maps
    res = run_bass_kernel_spmd(nc, in_maps, core_ids=list(range(8)))

    # output-projection bias applied on host (elementwise)
    y = np.empty((B, S, D), np.float32)
    for i, (b, gq) in enumerate(core_assign):
        y[b, gq * NQ : (gq + 1) * NQ] = (
            res.results[i]["y"].astype(np.float32) + bo_h
        )
    return y
